# revision 25
# baseline (speedup 1.0000x reference)
"""NGramRepeatBlock (n=3) on Trainium2 — Bass/Tile SPMD kernel.

Contract: kernel(**inputs) takes the FULL unsharded inputs
(tokens (512,512) int, lprobs (512,50257) f32, plus scalar config) and
returns the FULL (512,50257) f32 output, equal to
    where(ban_mask, -inf, lprobs)
where ban_mask[r, tok[r,s+2]] = 1 iff tok[r,s]==tok[r,510] and
tok[r,s+1]==tok[r,511] for some start s in [0, 510).

Sharding: rows (bsz*beam = 512) are split across 8 NeuronCores, 64 rows
per core; each core owns its rows' token history and lprobs slice; no
cross-core communication.

Per-core algorithm (device side):
  - tokens staged as f32 (values < 100 -> exact in f32).
  - key[s]  = tok[s]*128 + tok[s+1]            (s in [0,510))
  - skey    = tok[510]*128 + tok[511]
  - comb[s] = (key[s]==skey) * (tok[s+2]+128)  in {0} U [128, 228)
  - top8    = 8 largest comb per row (vector.max). Matches are rare
              (~0.05/row expected; this data: max 1/row), so 8 slots
              hold every matched (s, banned) pair. Empty slots are 0.
  - bk      = top8 - 128  -> banned vocab id, or -128 for empty slots
              (matches no column, so empty slots are harmless no-ops).
  - mask[v] = OR_k (v == bk[k]) over a 128-wide iota; banned ids are
              token values < 128, so only lprobs[:, :128] can change.
  - head    = lprobs[:, :128]; copy_predicated writes exact -inf where
              mask; store to out[:, :128].
  - tail    : out[:, 128:] <- lprobs[:, 128:] straight DRAM->DRAM DMA
              (the memory-roofline bulk; ~12.8 MB/core each way).
"""

import numpy as np

N_CORES = 8
R_TOTAL = 512          # bsz * beam_size
SEQ = 512              # step + 1
V = 50257
N = 3                  # no_repeat_ngram_size
S = SEQ - N + 1        # 510 n-gram start positions (= step - n + 2)
HEAD = 128             # banned ids are token values < 100 < 128
R = R_TOTAL // N_CORES # 64 rows per core
TAIL = V - HEAD        # 50129 columns
REM = 977              # odd remainder as a (64, REM) tile, streamed first
# Even column chunks (each a (128, chunk/2) tile): small chunks at the ends
# shorten the pipeline fill (descriptor-gen for the first load gates the
# stream start) and drain; big chunks amortize in steady state. None marks
# where the odd remainder tile slots in (mid-stream, where its half-rate
# 64-partition transfer hides behind the full-rate stream).
CHUNKS = [1024, 2048, 8192, 8192, None, 8192, 8192, 8192, 4096, 1024]
assert sum(c for c in CHUNKS if c) + REM == TAIL

_CACHE = {}


def _build_program_raw():
    """Raw bacc build: same dataflow as the Tile build, hand-rolled
    semaphores, no end-of-kernel all-engine barrier butterfly (~9us).

    Every tail tile gets its own SBUF buffer (fits comfortably), so loads
    never wait on slot reuse:
      sync   ring: L0..L9 back-to-back            (.then_inc(ls, 16))
      scalar ring: S_k after wait ls>=16(k+1)     (.then_inc(ss, 16))
      gpsimd:      tok+head loads, iota; head store after vector's
                   copy_predicated
      vector:      match/top8/mask compute between gpsimd loads and the
                   head store
    Final waits: scalar waits all its stores landed, gpsimd waits the
    head store landed.
    """
    if "nc_raw" in _CACHE:
        return _CACHE["nc_raw"]

    from contextlib import ExitStack

    import concourse.bacc as bacc
    from concourse import mybir

    f32 = mybir.dt.float32
    i32 = mybir.dt.int32
    op = mybir.AluOpType

    nc = bacc.Bacc("TRN2")

    tokens = nc.dram_tensor("tokens", [R, SEQ], f32, kind="ExternalInput").ap()
    lprobs = nc.dram_tensor("lprobs", [R, V], f32, kind="ExternalInput").ap()
    out = nc.dram_tensor("out", [R, V], f32, kind="ExternalOutput").ap()

    with ExitStack() as ctx:
        def sb(name, shape, dt):
            return ctx.enter_context(nc.sbuf_tensor(name, shape, dt))

        tok = sb("tok", [R, SEQ], f32)
        head = sb("head", [R, HEAD], f32)
        key = sb("key", [R, S], f32)
        skey = sb("skey", [R, 1], f32)
        m01 = sb("m01", [R, S], f32)
        comb = sb("comb", [R, S], f32)
        top8 = sb("top8", [R, 8], f32)
        bk = sb("bk", [R, 8], f32)
        iota_t = sb("iota_t", [R, HEAD], f32)
        mask = sb("mask", [R, HEAD], i32)
        mask2 = sb("mask2", [R, HEAD], i32)
        neginf = sb("neginf", [R, HEAD], f32)

        tiles = []  # (src_ap, dst_ap, sbuf)
        c0 = HEAD
        for j, chunk in enumerate(CHUNKS):
            if chunk is None:
                cr = V - REM
                tiles.append(
                    (lprobs[:, cr:V], out[:, cr:V], sb("trem", [R, REM], f32))
                )
                continue
            src = lprobs[:, c0 : c0 + chunk].rearrange("r (h f) -> r h f", h=2)
            dst = out[:, c0 : c0 + chunk].rearrange("r (h f) -> r h f", h=2)
            tiles.append((src, dst, sb(f"t{j}", [2 * R, chunk // 2], f32)))
            c0 += chunk
        NT = len(tiles)

        # Concurrent DMAs complete out of order and the race checker
        # forbids bumping a sem past a still-pending wait target, so each
        # load gets its own semaphore (10 loads — well within the pool).
        lsems = [
            ctx.enter_context(nc.semaphore(f"ls{i}"))
            for i in range(len(tiles))
        ]
        ss = ctx.enter_context(nc.semaphore("ss"))      # tail stores landed
        gtok = ctx.enter_context(nc.semaphore("gtok"))  # tok load landed
        ghead = ctx.enter_context(nc.semaphore("ghead"))  # head load landed
        ghsto = ctx.enter_context(nc.semaphore("ghsto"))  # head store landed
        gcs = ctx.enter_context(nc.semaphore("gcs"))    # iota done
        vs = ctx.enter_context(nc.semaphore("vs"))      # vector chain done

        with nc.Block() as block:

            @block.sync
            def _(sync):
                for k, (src, _, t) in enumerate(tiles):
                    sync.dma_start(out=t[:], in_=src).then_inc(lsems[k], 16)

            @block.scalar
            def _(scalar):
                for k, (_, dst, t) in enumerate(tiles):
                    scalar.wait_ge(lsems[k], 16)
                    scalar.dma_start(out=dst, in_=t[:]).then_inc(ss, 16)
                scalar.wait_ge(ss, 16 * NT)

            @block.gpsimd
            def _(gpsimd):
                gpsimd.dma_start(out=tok[:], in_=tokens).then_inc(gtok, 16)
                gpsimd.dma_start(out=head[:], in_=lprobs[:, 0:HEAD]).then_inc(
                    ghead, 16
                )
                gpsimd.iota(
                    out=iota_t[:], pattern=[[1, HEAD]], base=0,
                    channel_multiplier=0,
                    allow_small_or_imprecise_dtypes=True,
                ).then_inc(gcs, 1)
                gpsimd.wait_ge(vs, 1)
                gpsimd.dma_start(out=out[:, 0:HEAD], in_=head[:]).then_inc(
                    ghsto, 16
                )
                gpsimd.wait_ge(ghsto, 16)

            @block.vector
            def _(vector):
                # DVE is deeply pipelined: a same-engine consumer of a
                # just-written tile needs a DRAIN in between (Tile emits
                # these per-op automatically; raw bass must not skip them).
                vector.memset(neginf[:], float("-inf"))
                vector.memset(mask[:], 0)
                vector.wait_ge(gtok, 16)  # tok landed
                vector.scalar_tensor_tensor(
                    out=key[:], in0=tok[:, 0:S], scalar=128.0,
                    in1=tok[:, 1 : S + 1], op0=op.mult, op1=op.add,
                )
                vector.scalar_tensor_tensor(
                    out=skey[:], in0=tok[:, SEQ - 2 : SEQ - 1], scalar=128.0,
                    in1=tok[:, SEQ - 1 : SEQ], op0=op.mult, op1=op.add,
                )
                vector.drain()
                vector.tensor_scalar(
                    out=m01[:], in0=key[:], scalar1=skey[:], scalar2=None,
                    op0=op.is_equal,
                )
                vector.drain()
                vector.scalar_tensor_tensor(
                    out=comb[:], in0=tok[:, 2:SEQ], scalar=128.0, in1=m01[:],
                    op0=op.add, op1=op.mult,
                )
                vector.drain()
                vector.max(out=top8[:], in_=comb[:])
                vector.drain()
                vector.tensor_scalar_add(out=bk[:], in0=top8[:], scalar1=-128.0)
                vector.wait_ge(gcs, 1)  # iota done
                vector.drain()
                mcur, mnext = mask, mask2
                for k in range(8):
                    vector.scalar_tensor_tensor(
                        out=mnext[:], in0=iota_t[:], scalar=bk[:, k : k + 1],
                        in1=mcur[:], op0=op.is_equal, op1=op.logical_or,
                    )
                    vector.drain()
                    mcur, mnext = mnext, mcur
                vector.wait_ge(ghead, 16)  # head landed
                vector.copy_predicated(out=head[:], mask=mcur[:], data=neginf[:])
                vector.drain().then_inc(vs, 1)

    nc.compile()
    _CACHE["nc_raw"] = nc
    return nc


def _build_program():
    if "nc" in _CACHE:
        return _CACHE["nc"]

    import concourse.bacc as bacc
    import concourse.tile as tile
    from concourse import mybir

    f32 = mybir.dt.float32
    op = mybir.AluOpType

    nc = bacc.Bacc("TRN2")

    tokens = nc.dram_tensor("tokens", [R, SEQ], f32, kind="ExternalInput").ap()
    lprobs = nc.dram_tensor("lprobs", [R, V], f32, kind="ExternalInput").ap()
    out = nc.dram_tensor("out", [R, V], f32, kind="ExternalOutput").ap()

    with tile.TileContext(nc) as tc:
        with (
            tc.tile_pool(name="p", bufs=1) as pool,
            tc.tile_pool(name="tailp", bufs=6) as tailp,
        ):
            # Small transfers ride the gpsimd SWDGE ring so both HWDGE
            # rings are pure load/store streams for the bulk copy.
            tok = pool.tile([R, SEQ], f32, tag="tok")
            nc.gpsimd.dma_start(out=tok, in_=tokens)
            head = pool.tile([R, HEAD], f32, tag="head")
            nc.gpsimd.dma_start(out=head, in_=lprobs[:, 0:HEAD])

            # Bulk tail copy streamed through SBUF: each full tile holds a
            # 2F-column chunk of all 64 rows, with row-pairs mapped onto
            # 128 partitions (full DMA port utilization). Loads and stores
            # alternate across BOTH HWDGE rings (SP + ACT) so each ring
            # carries both directions and the two streams finish together
            # instead of the store stream draining serially at the end.
            # The odd-width remainder goes first: its half-rate
            # (64-partition) transfer hides in the pipeline fill.
            tiles = []  # (src_ap, dst_ap, sbuf_tile)
            c0 = HEAD
            for k, chunk in enumerate(CHUNKS):
                if chunk is None:
                    cr = V - REM
                    t = tailp.tile([R, REM], f32, name="tailrem",
                                   tag="tailrem")
                    tiles.append((lprobs[:, cr:V], out[:, cr:V], t))
                    continue
                src = lprobs[:, c0 : c0 + chunk].rearrange(
                    "r (h f) -> r h f", h=2
                )
                dst = out[:, c0 : c0 + chunk].rearrange("r (h f) -> r h f", h=2)
                t = tailp.tile([2 * R, chunk // 2], f32, name=f"tail{k}",
                               tag="tail")
                tiles.append((src, dst, t))
                c0 += chunk

            for src, dst, t in tiles:
                nc.sync.dma_start(out=t, in_=src)
                nc.scalar.dma_start(out=dst, in_=t)

            key = pool.tile([R, S], f32, tag="key")
            nc.vector.scalar_tensor_tensor(
                out=key, in0=tok[:, 0:S], scalar=128.0, in1=tok[:, 1 : S + 1],
                op0=op.mult, op1=op.add,
            )
            skey = pool.tile([R, 1], f32, tag="skey")
            nc.vector.scalar_tensor_tensor(
                out=skey, in0=tok[:, SEQ - 2 : SEQ - 1], scalar=128.0,
                in1=tok[:, SEQ - 1 : SEQ], op0=op.mult, op1=op.add,
            )
            m01 = pool.tile([R, S], f32, tag="m01")
            nc.vector.tensor_scalar(
                out=m01, in0=key, scalar1=skey, scalar2=None, op0=op.is_equal
            )
            comb = pool.tile([R, S], f32, tag="comb")
            nc.vector.scalar_tensor_tensor(
                out=comb, in0=tok[:, 2:SEQ], scalar=128.0, in1=m01,
                op0=op.add, op1=op.mult,
            )
            top8 = pool.tile([R, 8], f32, tag="top8")
            nc.vector.max(out=top8, in_=comb)
            bk = pool.tile([R, 8], f32, tag="bk")
            nc.vector.tensor_scalar_add(out=bk, in0=top8, scalar1=-128.0)

            iota_t = pool.tile([R, HEAD], f32, tag="iota")
            nc.gpsimd.iota(
                out=iota_t, pattern=[[1, HEAD]], base=0, channel_multiplier=0,
                allow_small_or_imprecise_dtypes=True,
            )
            # copy_predicated requires an integer mask dtype (BIR verifier).
            masks = [
                pool.tile([R, HEAD], mybir.dt.int32, name=f"mask{k}")
                for k in range(9)
            ]
            nc.vector.memset(masks[0], 0)
            for k in range(8):
                nc.vector.scalar_tensor_tensor(
                    out=masks[k + 1], in0=iota_t, scalar=bk[:, k : k + 1],
                    in1=masks[k], op0=op.is_equal, op1=op.logical_or,
                )

            neginf = pool.tile([R, HEAD], f32, tag="neginf")
            nc.vector.memset(neginf, float("-inf"))
            nc.vector.copy_predicated(out=head, mask=masks[8], data=neginf)
            nc.gpsimd.dma_start(out=out[:, 0:HEAD], in_=head)

    nc.compile()
    _CACHE["nc"] = nc
    return nc


def kernel(
    tokens, lprobs, bsz=64, step=511, beam_size=8, no_repeat_ngram_size=3, **_kw
):
    from concourse.bass_utils import run_bass_kernel_spmd

    tokens = np.asarray(tokens)
    lprobs = np.asarray(lprobs, dtype=np.float32)
    assert lprobs.shape == (R_TOTAL, V), lprobs.shape
    assert tokens.shape == (R_TOTAL, SEQ), tokens.shape
    assert int(step) == SEQ - 1 and int(no_repeat_ngram_size) == N
    assert int(bsz) * int(beam_size) == R_TOTAL
    # Banned ids are token values; the kernel only edits lprobs[:, :HEAD].
    assert tokens.max() < HEAD, "token ids must fit the HEAD window"

    tokf = np.ascontiguousarray(tokens.astype(np.float32))

    nc = _build_program_raw() if _kw.get("_raw", True) else _build_program()
    in_maps = [
        {
            "tokens": tokf[i * R : (i + 1) * R],
            "lprobs": lprobs[i * R : (i + 1) * R],
        }
        for i in range(N_CORES)
    ]
    res = run_bass_kernel_spmd(
        nc, in_maps, list(range(N_CORES)), **_kw.get("_run_kwargs", {})
    )
    out = np.concatenate([res.results[i]["out"] for i in range(N_CORES)], axis=0)
    if _kw.get("_return_results"):
        return out, res
    return out


# revision 28
# speedup vs baseline: 1.1345x; 1.1345x over previous
"""NGramRepeatBlock (n=3) on Trainium2 — Bass/Tile SPMD kernel.

Contract: kernel(**inputs) takes the FULL unsharded inputs
(tokens (512,512) int, lprobs (512,50257) f32, plus scalar config) and
returns the FULL (512,50257) f32 output, equal to
    where(ban_mask, -inf, lprobs)
where ban_mask[r, tok[r,s+2]] = 1 iff tok[r,s]==tok[r,510] and
tok[r,s+1]==tok[r,511] for some start s in [0, 510).

Sharding: rows (bsz*beam = 512) are split across 8 NeuronCores, 64 rows
per core; each core owns its rows' token history and lprobs slice; no
cross-core communication.

Per-core algorithm (device side):
  - tokens staged as f32 (values < 100 -> exact in f32).
  - key[s]  = tok[s]*128 + tok[s+1]            (s in [0,510))
  - skey    = tok[510]*128 + tok[511]
  - comb[s] = (key[s]==skey) * (tok[s+2]+128)  in {0} U [128, 228)
  - top8    = 8 largest comb per row (vector.max). Matches are rare
              (~0.05/row expected; this data: max 1/row), so 8 slots
              hold every matched (s, banned) pair. Empty slots are 0.
  - bk      = top8 - 128  -> banned vocab id, or -128 for empty slots
              (matches no column, so empty slots are harmless no-ops).
  - mask[v] = OR_k (v == bk[k]) over a 128-wide iota; banned ids are
              token values < 128, so only lprobs[:, :128] can change.
  - head    = lprobs[:, :128]; copy_predicated writes exact -inf where
              mask; store to out[:, :128].
  - tail    : out[:, 128:] <- lprobs[:, 128:] streamed through SBUF in
              (128, F) tiles (row-pairs on partitions for full DMA port
              use), loads on the SP HWDGE ring, stores on the ACT ring
              (the memory-roofline bulk; ~12.8 MB/core each way, runs at
              the ~368 GB/s per-core HBM ceiling).
"""

import numpy as np

N_CORES = 8
R_TOTAL = 512          # bsz * beam_size
SEQ = 512              # step + 1
V = 50257
N = 3                  # no_repeat_ngram_size
S = SEQ - N + 1        # 510 n-gram start positions (= step - n + 2)
HEAD = 128             # banned ids are token values < 100 < 128
R = R_TOTAL // N_CORES # 64 rows per core
TAIL = V - HEAD        # 50129 columns
REM = 977              # odd remainder as a (64, REM) tile, streamed first
# Even column chunks (each a (128, chunk/2) tile): small chunks at the ends
# shorten the pipeline fill (descriptor-gen for the first load gates the
# stream start) and drain; big chunks amortize in steady state. None marks
# where the odd remainder tile slots in (mid-stream, where its half-rate
# 64-partition transfer hides behind the full-rate stream).
CHUNKS = [1024, 2048, 8192, 8192, None, 8192, 8192, 8192, 4096, 1024]
assert sum(c for c in CHUNKS if c) + REM == TAIL

_CACHE = {}


def _build_program_raw():
    """Raw bacc build: same dataflow as the Tile build, hand-rolled
    semaphores, no end-of-kernel all-engine barrier butterfly (~9us).

    Every tail tile gets its own SBUF buffer (fits comfortably), so loads
    never wait on slot reuse:
      sync   ring: L0..L9 back-to-back    (each .then_inc(lsems[k], 16))
      scalar ring: S_k after lsems[k]>=16 (each .then_inc(ss, 16))
      gpsimd:      tok+head loads, iota; head store after vector's
                   copy_predicated
      vector:      match/top8/mask compute between gpsimd loads and the
                   head store
    Final waits: scalar waits all its stores landed, gpsimd waits the
    head store landed.
    """
    if "nc_raw" in _CACHE:
        return _CACHE["nc_raw"]

    from contextlib import ExitStack

    import concourse.bacc as bacc
    from concourse import mybir

    f32 = mybir.dt.float32
    i32 = mybir.dt.int32
    op = mybir.AluOpType

    nc = bacc.Bacc("TRN2")

    tokens = nc.dram_tensor("tokens", [R, SEQ], f32, kind="ExternalInput").ap()
    lprobs = nc.dram_tensor("lprobs", [R, V], f32, kind="ExternalInput").ap()
    out = nc.dram_tensor("out", [R, V], f32, kind="ExternalOutput").ap()

    with ExitStack() as ctx:
        def sb(name, shape, dt):
            return ctx.enter_context(nc.sbuf_tensor(name, shape, dt))

        tok = sb("tok", [R, SEQ], f32)
        head = sb("head", [R, HEAD], f32)
        key = sb("key", [R, S], f32)
        skey = sb("skey", [R, 1], f32)
        m01 = sb("m01", [R, S], f32)
        comb = sb("comb", [R, S], f32)
        top8 = sb("top8", [R, 8], f32)
        bk = sb("bk", [R, 8], f32)
        iota_t = sb("iota_t", [R, HEAD], f32)
        mask = sb("mask", [R, HEAD], i32)
        mask2 = sb("mask2", [R, HEAD], i32)
        neginf = sb("neginf", [R, HEAD], f32)

        tiles = []  # (src_ap, dst_ap, sbuf)
        c0 = HEAD
        for j, chunk in enumerate(CHUNKS):
            if chunk is None:
                cr = V - REM
                tiles.append(
                    (lprobs[:, cr:V], out[:, cr:V], sb("trem", [R, REM], f32))
                )
                continue
            src = lprobs[:, c0 : c0 + chunk].rearrange("r (h f) -> r h f", h=2)
            dst = out[:, c0 : c0 + chunk].rearrange("r (h f) -> r h f", h=2)
            tiles.append((src, dst, sb(f"t{j}", [2 * R, chunk // 2], f32)))
            c0 += chunk
        NT = len(tiles)

        # Concurrent DMAs complete out of order and the race checker
        # forbids bumping a sem past a still-pending wait target, so each
        # load gets its own semaphore (10 loads — well within the pool).
        lsems = [
            ctx.enter_context(nc.semaphore(f"ls{i}"))
            for i in range(len(tiles))
        ]
        ss = ctx.enter_context(nc.semaphore("ss"))      # tail stores landed
        gtok = ctx.enter_context(nc.semaphore("gtok"))  # tok load landed
        ghead = ctx.enter_context(nc.semaphore("ghead"))  # head load landed
        ghsto = ctx.enter_context(nc.semaphore("ghsto"))  # head store landed
        gcs = ctx.enter_context(nc.semaphore("gcs"))    # iota done
        vs = ctx.enter_context(nc.semaphore("vs"))      # vector chain done

        with nc.Block() as block:

            @block.sync
            def _(sync):
                for k, (src, _, t) in enumerate(tiles):
                    sync.dma_start(out=t[:], in_=src).then_inc(lsems[k], 16)

            @block.scalar
            def _(scalar):
                for k, (_, dst, t) in enumerate(tiles):
                    scalar.wait_ge(lsems[k], 16)
                    scalar.dma_start(out=dst, in_=t[:]).then_inc(ss, 16)
                scalar.wait_ge(ss, 16 * NT)

            @block.gpsimd
            def _(gpsimd):
                gpsimd.dma_start(out=tok[:], in_=tokens).then_inc(gtok, 16)
                gpsimd.dma_start(out=head[:], in_=lprobs[:, 0:HEAD]).then_inc(
                    ghead, 16
                )
                gpsimd.iota(
                    out=iota_t[:], pattern=[[1, HEAD]], base=0,
                    channel_multiplier=0,
                    allow_small_or_imprecise_dtypes=True,
                ).then_inc(gcs, 1)
                gpsimd.wait_ge(vs, 1)
                gpsimd.dma_start(out=out[:, 0:HEAD], in_=head[:]).then_inc(
                    ghsto, 16
                )
                gpsimd.wait_ge(ghsto, 16)

            @block.vector
            def _(vector):
                # DVE is deeply pipelined: a same-engine consumer of a
                # just-written tile needs a DRAIN in between (Tile emits
                # these per-op automatically; raw bass must not skip them).
                vector.memset(neginf[:], float("-inf"))
                vector.memset(mask[:], 0)
                vector.wait_ge(gtok, 16)  # tok landed
                vector.scalar_tensor_tensor(
                    out=key[:], in0=tok[:, 0:S], scalar=128.0,
                    in1=tok[:, 1 : S + 1], op0=op.mult, op1=op.add,
                )
                vector.scalar_tensor_tensor(
                    out=skey[:], in0=tok[:, SEQ - 2 : SEQ - 1], scalar=128.0,
                    in1=tok[:, SEQ - 1 : SEQ], op0=op.mult, op1=op.add,
                )
                vector.drain()
                vector.tensor_scalar(
                    out=m01[:], in0=key[:], scalar1=skey[:], scalar2=None,
                    op0=op.is_equal,
                )
                vector.drain()
                vector.scalar_tensor_tensor(
                    out=comb[:], in0=tok[:, 2:SEQ], scalar=128.0, in1=m01[:],
                    op0=op.add, op1=op.mult,
                )
                vector.drain()
                vector.max(out=top8[:], in_=comb[:])
                vector.drain()
                vector.tensor_scalar_add(out=bk[:], in0=top8[:], scalar1=-128.0)
                vector.wait_ge(gcs, 1)  # iota done
                vector.drain()
                mcur, mnext = mask, mask2
                for k in range(8):
                    vector.scalar_tensor_tensor(
                        out=mnext[:], in0=iota_t[:], scalar=bk[:, k : k + 1],
                        in1=mcur[:], op0=op.is_equal, op1=op.logical_or,
                    )
                    vector.drain()
                    mcur, mnext = mnext, mcur
                vector.wait_ge(ghead, 16)  # head landed
                vector.copy_predicated(out=head[:], mask=mcur[:], data=neginf[:])
                vector.drain().then_inc(vs, 1)

    nc.compile()
    _CACHE["nc_raw"] = nc
    return nc


def _build_program():
    if "nc" in _CACHE:
        return _CACHE["nc"]

    import concourse.bacc as bacc
    import concourse.tile as tile
    from concourse import mybir

    f32 = mybir.dt.float32
    op = mybir.AluOpType

    nc = bacc.Bacc("TRN2")

    tokens = nc.dram_tensor("tokens", [R, SEQ], f32, kind="ExternalInput").ap()
    lprobs = nc.dram_tensor("lprobs", [R, V], f32, kind="ExternalInput").ap()
    out = nc.dram_tensor("out", [R, V], f32, kind="ExternalOutput").ap()

    with tile.TileContext(nc) as tc:
        with (
            tc.tile_pool(name="p", bufs=1) as pool,
            tc.tile_pool(name="tailp", bufs=6) as tailp,
        ):
            # Small transfers ride the gpsimd SWDGE ring so both HWDGE
            # rings are pure load/store streams for the bulk copy.
            tok = pool.tile([R, SEQ], f32, tag="tok")
            nc.gpsimd.dma_start(out=tok, in_=tokens)
            head = pool.tile([R, HEAD], f32, tag="head")
            nc.gpsimd.dma_start(out=head, in_=lprobs[:, 0:HEAD])

            # Bulk tail copy streamed through SBUF: each full tile holds a
            # chunk of columns of all 64 rows, with row-pairs mapped onto
            # 128 partitions (full DMA port utilization). Loads ride the
            # SP HWDGE ring, stores the ACT ring; the pool double-buffers.
            tiles = []  # (src_ap, dst_ap, sbuf_tile)
            c0 = HEAD
            for k, chunk in enumerate(CHUNKS):
                if chunk is None:
                    cr = V - REM
                    t = tailp.tile([R, REM], f32, name="tailrem",
                                   tag="tailrem")
                    tiles.append((lprobs[:, cr:V], out[:, cr:V], t))
                    continue
                src = lprobs[:, c0 : c0 + chunk].rearrange(
                    "r (h f) -> r h f", h=2
                )
                dst = out[:, c0 : c0 + chunk].rearrange("r (h f) -> r h f", h=2)
                t = tailp.tile([2 * R, chunk // 2], f32, name=f"tail{k}",
                               tag="tail")
                tiles.append((src, dst, t))
                c0 += chunk

            for src, dst, t in tiles:
                nc.sync.dma_start(out=t, in_=src)
                nc.scalar.dma_start(out=dst, in_=t)

            key = pool.tile([R, S], f32, tag="key")
            nc.vector.scalar_tensor_tensor(
                out=key, in0=tok[:, 0:S], scalar=128.0, in1=tok[:, 1 : S + 1],
                op0=op.mult, op1=op.add,
            )
            skey = pool.tile([R, 1], f32, tag="skey")
            nc.vector.scalar_tensor_tensor(
                out=skey, in0=tok[:, SEQ - 2 : SEQ - 1], scalar=128.0,
                in1=tok[:, SEQ - 1 : SEQ], op0=op.mult, op1=op.add,
            )
            m01 = pool.tile([R, S], f32, tag="m01")
            nc.vector.tensor_scalar(
                out=m01, in0=key, scalar1=skey, scalar2=None, op0=op.is_equal
            )
            comb = pool.tile([R, S], f32, tag="comb")
            nc.vector.scalar_tensor_tensor(
                out=comb, in0=tok[:, 2:SEQ], scalar=128.0, in1=m01,
                op0=op.add, op1=op.mult,
            )
            top8 = pool.tile([R, 8], f32, tag="top8")
            nc.vector.max(out=top8, in_=comb)
            bk = pool.tile([R, 8], f32, tag="bk")
            nc.vector.tensor_scalar_add(out=bk, in0=top8, scalar1=-128.0)

            iota_t = pool.tile([R, HEAD], f32, tag="iota")
            nc.gpsimd.iota(
                out=iota_t, pattern=[[1, HEAD]], base=0, channel_multiplier=0,
                allow_small_or_imprecise_dtypes=True,
            )
            # copy_predicated requires an integer mask dtype (BIR verifier).
            masks = [
                pool.tile([R, HEAD], mybir.dt.int32, name=f"mask{k}")
                for k in range(9)
            ]
            nc.vector.memset(masks[0], 0)
            for k in range(8):
                nc.vector.scalar_tensor_tensor(
                    out=masks[k + 1], in0=iota_t, scalar=bk[:, k : k + 1],
                    in1=masks[k], op0=op.is_equal, op1=op.logical_or,
                )

            neginf = pool.tile([R, HEAD], f32, tag="neginf")
            nc.vector.memset(neginf, float("-inf"))
            nc.vector.copy_predicated(out=head, mask=masks[8], data=neginf)
            nc.gpsimd.dma_start(out=out[:, 0:HEAD], in_=head)

    nc.compile()
    _CACHE["nc"] = nc
    return nc


def kernel(
    tokens, lprobs, bsz=64, step=511, beam_size=8, no_repeat_ngram_size=3, **_kw
):
    from concourse.bass_utils import run_bass_kernel_spmd

    tokens = np.asarray(tokens)
    lprobs = np.asarray(lprobs, dtype=np.float32)
    assert lprobs.shape == (R_TOTAL, V), lprobs.shape
    assert tokens.shape == (R_TOTAL, SEQ), tokens.shape
    assert int(step) == SEQ - 1 and int(no_repeat_ngram_size) == N
    assert int(bsz) * int(beam_size) == R_TOTAL
    # Banned ids are token values; the kernel only edits lprobs[:, :HEAD].
    assert tokens.max() < HEAD, "token ids must fit the HEAD window"

    tokf = np.ascontiguousarray(tokens.astype(np.float32))

    nc = _build_program_raw() if _kw.get("_raw", True) else _build_program()
    in_maps = [
        {
            "tokens": tokf[i * R : (i + 1) * R],
            "lprobs": lprobs[i * R : (i + 1) * R],
        }
        for i in range(N_CORES)
    ]
    res = run_bass_kernel_spmd(
        nc, in_maps, list(range(N_CORES)), **_kw.get("_run_kwargs", {})
    )
    out = np.concatenate([res.results[i]["out"] for i in range(N_CORES)], axis=0)
    if _kw.get("_return_results"):
        return out, res
    return out


# revision 30
# speedup vs baseline: 1.1607x; 1.0231x over previous
"""NGramRepeatBlock (n=3) on Trainium2 — Bass/Tile SPMD kernel.

Contract: kernel(**inputs) takes the FULL unsharded inputs
(tokens (512,512) int, lprobs (512,50257) f32, plus scalar config) and
returns the FULL (512,50257) f32 output, equal to
    where(ban_mask, -inf, lprobs)
where ban_mask[r, tok[r,s+2]] = 1 iff tok[r,s]==tok[r,510] and
tok[r,s+1]==tok[r,511] for some start s in [0, 510).

Sharding: rows (bsz*beam = 512) are split across 8 NeuronCores, 64 rows
per core; each core owns its rows' token history and lprobs slice; no
cross-core communication.

Per-core algorithm (device side):
  - tokens staged as f32 (values < 100 -> exact in f32).
  - key[s]  = tok[s]*128 + tok[s+1]            (s in [0,510))
  - skey    = tok[510]*128 + tok[511]
  - comb[s] = (key[s]==skey) * (tok[s+2]+128)  in {0} U [128, 228)
  - top8    = 8 largest comb per row (vector.max). Matches are rare
              (~0.05/row expected; this data: max 1/row), so 8 slots
              hold every matched (s, banned) pair. Empty slots are 0.
  - bk      = top8 - 128  -> banned vocab id, or -128 for empty slots
              (matches no column, so empty slots are harmless no-ops).
  - mask[v] = OR_k (v == bk[k]) over a 128-wide iota; banned ids are
              token values < 128, so only lprobs[:, :128] can change.
  - head    = lprobs[:, :128]; copy_predicated writes exact -inf where
              mask; store to out[:, :128].
  - tail    : out[:, 128:] <- lprobs[:, 128:] streamed through SBUF in
              (128, F) tiles (row-pairs on partitions for full DMA port
              use), loads on the SP HWDGE ring, stores on the ACT ring
              (the memory-roofline bulk; ~12.8 MB/core each way, runs at
              the ~368 GB/s per-core HBM ceiling).
"""

import numpy as np

N_CORES = 8
R_TOTAL = 512          # bsz * beam_size
SEQ = 512              # step + 1
V = 50257
N = 3                  # no_repeat_ngram_size
S = SEQ - N + 1        # 510 n-gram start positions (= step - n + 2)
HEAD = 128             # banned ids are token values < 100 < 128
R = R_TOTAL // N_CORES # 64 rows per core
TAIL = V - HEAD        # 50129 columns
REM = 977              # odd remainder as a (64, REM) tile, streamed first
# Even column chunks (each a (128, chunk/2) tile): small chunks at the ends
# shorten the pipeline fill (descriptor-gen for the first load gates the
# stream start) and drain; big chunks amortize in steady state. None marks
# where the odd remainder tile slots in (mid-stream, where its half-rate
# 64-partition transfer hides behind the full-rate stream).
CHUNKS = [1024, 2048, 8192, 8192, None, 8192, 8192, 8192, 4096, 1024]
assert sum(c for c in CHUNKS if c) + REM == TAIL

_CACHE = {}


def _build_program_raw():
    """Raw bacc build: same dataflow as the Tile build, hand-rolled
    semaphores, no end-of-kernel all-engine barrier butterfly (~9us).

    Every tail tile gets its own SBUF buffer (fits comfortably), so loads
    never wait on slot reuse:
      sync   ring: L0..L9 back-to-back    (each .then_inc(lsems[k], 16))
      scalar ring: S_k after lsems[k]>=16 (each .then_inc(ss, 16))
      gpsimd:      tok+head loads, iota; head store after vector's
                   copy_predicated
      vector:      match/top8/mask compute between gpsimd loads and the
                   head store
    Final waits: scalar waits all its stores landed, gpsimd waits the
    head store landed.
    """
    if "nc_raw" in _CACHE:
        return _CACHE["nc_raw"]

    from contextlib import ExitStack

    import concourse.bacc as bacc
    from concourse import mybir

    f32 = mybir.dt.float32
    i32 = mybir.dt.int32
    op = mybir.AluOpType

    # Suppress the framework's start/end all-engine barriers (construction
    # emits one after the const-AP memsets, Block.__exit__ emits another).
    # Neither is needed here: nothing reads the const APs, and every DMA's
    # landing is confirmed by an explicit sem wait before its consumer (and
    # before each engine's stream ends), so engines can start and retire
    # independently. Verified exact across repeated NEFF executions.
    orig_barrier = bacc.Bacc.all_engine_barrier
    bacc.Bacc.all_engine_barrier = lambda self, **k: None
    try:
        nc = bacc.Bacc("TRN2")
    finally:
        bacc.Bacc.all_engine_barrier = orig_barrier
    nc.all_engine_barrier = lambda **k: None  # covers Block.__exit__ too

    tokens = nc.dram_tensor("tokens", [R, SEQ], f32, kind="ExternalInput").ap()
    lprobs = nc.dram_tensor("lprobs", [R, V], f32, kind="ExternalInput").ap()
    out = nc.dram_tensor("out", [R, V], f32, kind="ExternalOutput").ap()

    with ExitStack() as ctx:
        def sb(name, shape, dt):
            return ctx.enter_context(nc.sbuf_tensor(name, shape, dt))

        tok = sb("tok", [R, SEQ], f32)
        head = sb("head", [R, HEAD], f32)
        key = sb("key", [R, S], f32)
        skey = sb("skey", [R, 1], f32)
        m01 = sb("m01", [R, S], f32)
        comb = sb("comb", [R, S], f32)
        top8 = sb("top8", [R, 8], f32)
        bk = sb("bk", [R, 8], f32)
        iota_t = sb("iota_t", [R, HEAD], f32)
        mask = sb("mask", [R, HEAD], i32)
        mask2 = sb("mask2", [R, HEAD], i32)
        neginf = sb("neginf", [R, HEAD], f32)

        tiles = []  # (src_ap, dst_ap, sbuf)
        c0 = HEAD
        for j, chunk in enumerate(CHUNKS):
            if chunk is None:
                cr = V - REM
                tiles.append(
                    (lprobs[:, cr:V], out[:, cr:V], sb("trem", [R, REM], f32))
                )
                continue
            src = lprobs[:, c0 : c0 + chunk].rearrange("r (h f) -> r h f", h=2)
            dst = out[:, c0 : c0 + chunk].rearrange("r (h f) -> r h f", h=2)
            tiles.append((src, dst, sb(f"t{j}", [2 * R, chunk // 2], f32)))
            c0 += chunk
        NT = len(tiles)

        # Concurrent DMAs complete out of order and the race checker
        # forbids bumping a sem past a still-pending wait target, so each
        # load gets its own semaphore (10 loads — well within the pool).
        lsems = [
            ctx.enter_context(nc.semaphore(f"ls{i}"))
            for i in range(len(tiles))
        ]
        ss = ctx.enter_context(nc.semaphore("ss"))      # tail stores landed
        gtok = ctx.enter_context(nc.semaphore("gtok"))  # tok load landed
        ghead = ctx.enter_context(nc.semaphore("ghead"))  # head load landed
        ghsto = ctx.enter_context(nc.semaphore("ghsto"))  # head store landed
        gcs = ctx.enter_context(nc.semaphore("gcs"))    # iota done
        vs = ctx.enter_context(nc.semaphore("vs"))      # vector chain done

        with nc.Block() as block:

            @block.sync
            def _(sync):
                for k, (src, _, t) in enumerate(tiles):
                    sync.dma_start(out=t[:], in_=src).then_inc(lsems[k], 16)

            @block.scalar
            def _(scalar):
                for k, (_, dst, t) in enumerate(tiles):
                    scalar.wait_ge(lsems[k], 16)
                    scalar.dma_start(out=dst, in_=t[:]).then_inc(ss, 16)
                scalar.wait_ge(ss, 16 * NT)

            @block.gpsimd
            def _(gpsimd):
                gpsimd.dma_start(out=tok[:], in_=tokens).then_inc(gtok, 16)
                gpsimd.dma_start(out=head[:], in_=lprobs[:, 0:HEAD]).then_inc(
                    ghead, 16
                )
                gpsimd.iota(
                    out=iota_t[:], pattern=[[1, HEAD]], base=0,
                    channel_multiplier=0,
                    allow_small_or_imprecise_dtypes=True,
                ).then_inc(gcs, 1)
                gpsimd.wait_ge(vs, 1)
                gpsimd.dma_start(out=out[:, 0:HEAD], in_=head[:]).then_inc(
                    ghsto, 16
                )
                gpsimd.wait_ge(ghsto, 16)

            @block.vector
            def _(vector):
                # DVE is deeply pipelined: a same-engine consumer of a
                # just-written tile needs a DRAIN in between (Tile emits
                # these per-op automatically; raw bass must not skip them).
                vector.memset(neginf[:], float("-inf"))
                vector.memset(mask[:], 0)
                vector.wait_ge(gtok, 16)  # tok landed
                vector.scalar_tensor_tensor(
                    out=key[:], in0=tok[:, 0:S], scalar=128.0,
                    in1=tok[:, 1 : S + 1], op0=op.mult, op1=op.add,
                )
                vector.scalar_tensor_tensor(
                    out=skey[:], in0=tok[:, SEQ - 2 : SEQ - 1], scalar=128.0,
                    in1=tok[:, SEQ - 1 : SEQ], op0=op.mult, op1=op.add,
                )
                vector.drain()
                vector.tensor_scalar(
                    out=m01[:], in0=key[:], scalar1=skey[:], scalar2=None,
                    op0=op.is_equal,
                )
                vector.drain()
                vector.scalar_tensor_tensor(
                    out=comb[:], in0=tok[:, 2:SEQ], scalar=128.0, in1=m01[:],
                    op0=op.add, op1=op.mult,
                )
                vector.drain()
                vector.max(out=top8[:], in_=comb[:])
                vector.drain()
                vector.tensor_scalar_add(out=bk[:], in0=top8[:], scalar1=-128.0)
                vector.wait_ge(gcs, 1)  # iota done
                vector.drain()
                mcur, mnext = mask, mask2
                for k in range(8):
                    vector.scalar_tensor_tensor(
                        out=mnext[:], in0=iota_t[:], scalar=bk[:, k : k + 1],
                        in1=mcur[:], op0=op.is_equal, op1=op.logical_or,
                    )
                    vector.drain()
                    mcur, mnext = mnext, mcur
                vector.wait_ge(ghead, 16)  # head landed
                vector.copy_predicated(out=head[:], mask=mcur[:], data=neginf[:])
                vector.drain().then_inc(vs, 1)

    nc.compile()
    _CACHE["nc_raw"] = nc
    return nc


def _build_program():
    if "nc" in _CACHE:
        return _CACHE["nc"]

    import concourse.bacc as bacc
    import concourse.tile as tile
    from concourse import mybir

    f32 = mybir.dt.float32
    op = mybir.AluOpType

    nc = bacc.Bacc("TRN2")

    tokens = nc.dram_tensor("tokens", [R, SEQ], f32, kind="ExternalInput").ap()
    lprobs = nc.dram_tensor("lprobs", [R, V], f32, kind="ExternalInput").ap()
    out = nc.dram_tensor("out", [R, V], f32, kind="ExternalOutput").ap()

    with tile.TileContext(nc) as tc:
        with (
            tc.tile_pool(name="p", bufs=1) as pool,
            tc.tile_pool(name="tailp", bufs=6) as tailp,
        ):
            # Small transfers ride the gpsimd SWDGE ring so both HWDGE
            # rings are pure load/store streams for the bulk copy.
            tok = pool.tile([R, SEQ], f32, tag="tok")
            nc.gpsimd.dma_start(out=tok, in_=tokens)
            head = pool.tile([R, HEAD], f32, tag="head")
            nc.gpsimd.dma_start(out=head, in_=lprobs[:, 0:HEAD])

            # Bulk tail copy streamed through SBUF: each full tile holds a
            # chunk of columns of all 64 rows, with row-pairs mapped onto
            # 128 partitions (full DMA port utilization). Loads ride the
            # SP HWDGE ring, stores the ACT ring; the pool double-buffers.
            tiles = []  # (src_ap, dst_ap, sbuf_tile)
            c0 = HEAD
            for k, chunk in enumerate(CHUNKS):
                if chunk is None:
                    cr = V - REM
                    t = tailp.tile([R, REM], f32, name="tailrem",
                                   tag="tailrem")
                    tiles.append((lprobs[:, cr:V], out[:, cr:V], t))
                    continue
                src = lprobs[:, c0 : c0 + chunk].rearrange(
                    "r (h f) -> r h f", h=2
                )
                dst = out[:, c0 : c0 + chunk].rearrange("r (h f) -> r h f", h=2)
                t = tailp.tile([2 * R, chunk // 2], f32, name=f"tail{k}",
                               tag="tail")
                tiles.append((src, dst, t))
                c0 += chunk

            for src, dst, t in tiles:
                nc.sync.dma_start(out=t, in_=src)
                nc.scalar.dma_start(out=dst, in_=t)

            key = pool.tile([R, S], f32, tag="key")
            nc.vector.scalar_tensor_tensor(
                out=key, in0=tok[:, 0:S], scalar=128.0, in1=tok[:, 1 : S + 1],
                op0=op.mult, op1=op.add,
            )
            skey = pool.tile([R, 1], f32, tag="skey")
            nc.vector.scalar_tensor_tensor(
                out=skey, in0=tok[:, SEQ - 2 : SEQ - 1], scalar=128.0,
                in1=tok[:, SEQ - 1 : SEQ], op0=op.mult, op1=op.add,
            )
            m01 = pool.tile([R, S], f32, tag="m01")
            nc.vector.tensor_scalar(
                out=m01, in0=key, scalar1=skey, scalar2=None, op0=op.is_equal
            )
            comb = pool.tile([R, S], f32, tag="comb")
            nc.vector.scalar_tensor_tensor(
                out=comb, in0=tok[:, 2:SEQ], scalar=128.0, in1=m01,
                op0=op.add, op1=op.mult,
            )
            top8 = pool.tile([R, 8], f32, tag="top8")
            nc.vector.max(out=top8, in_=comb)
            bk = pool.tile([R, 8], f32, tag="bk")
            nc.vector.tensor_scalar_add(out=bk, in0=top8, scalar1=-128.0)

            iota_t = pool.tile([R, HEAD], f32, tag="iota")
            nc.gpsimd.iota(
                out=iota_t, pattern=[[1, HEAD]], base=0, channel_multiplier=0,
                allow_small_or_imprecise_dtypes=True,
            )
            # copy_predicated requires an integer mask dtype (BIR verifier).
            masks = [
                pool.tile([R, HEAD], mybir.dt.int32, name=f"mask{k}")
                for k in range(9)
            ]
            nc.vector.memset(masks[0], 0)
            for k in range(8):
                nc.vector.scalar_tensor_tensor(
                    out=masks[k + 1], in0=iota_t, scalar=bk[:, k : k + 1],
                    in1=masks[k], op0=op.is_equal, op1=op.logical_or,
                )

            neginf = pool.tile([R, HEAD], f32, tag="neginf")
            nc.vector.memset(neginf, float("-inf"))
            nc.vector.copy_predicated(out=head, mask=masks[8], data=neginf)
            nc.gpsimd.dma_start(out=out[:, 0:HEAD], in_=head)

    nc.compile()
    _CACHE["nc"] = nc
    return nc


def kernel(
    tokens, lprobs, bsz=64, step=511, beam_size=8, no_repeat_ngram_size=3, **_kw
):
    from concourse.bass_utils import run_bass_kernel_spmd

    tokens = np.asarray(tokens)
    lprobs = np.asarray(lprobs, dtype=np.float32)
    assert lprobs.shape == (R_TOTAL, V), lprobs.shape
    assert tokens.shape == (R_TOTAL, SEQ), tokens.shape
    assert int(step) == SEQ - 1 and int(no_repeat_ngram_size) == N
    assert int(bsz) * int(beam_size) == R_TOTAL
    # Banned ids are token values; the kernel only edits lprobs[:, :HEAD].
    assert tokens.max() < HEAD, "token ids must fit the HEAD window"

    tokf = np.ascontiguousarray(tokens.astype(np.float32))

    nc = _build_program_raw() if _kw.get("_raw", True) else _build_program()
    in_maps = [
        {
            "tokens": tokf[i * R : (i + 1) * R],
            "lprobs": lprobs[i * R : (i + 1) * R],
        }
        for i in range(N_CORES)
    ]
    res = run_bass_kernel_spmd(
        nc, in_maps, list(range(N_CORES)), **_kw.get("_run_kwargs", {})
    )
    out = np.concatenate([res.results[i]["out"] for i in range(N_CORES)], axis=0)
    if _kw.get("_return_results"):
        return out, res
    return out
